# revision 1
# baseline (speedup 1.0000x reference)
"""Binary linear layer (sign(x) @ sign(w)) on 8 trn2 NeuronCores.

Strategy
--------
Data-parallel: x is split into 8 row-blocks of 1024; the 4096x4096 weight is
replicated. Each core computes out_shard = sign(x_shard) @ sign(w).

All products are +/-1 and row sums are integers <= 4096, so the matmul is
exact in low precision with fp32 PSUM accumulation. The fast path ("fp8dr"):

- Host re-encodes both inputs to fp8e4 (sign-exact for every input value --
  see _encode_fp8) and pre-transposes each x shard to [d_in, n_per] so the
  PE contraction dim lands on SBUF partitions. 21 MB HBM in per core.
- Device binarizes x -> +/-1 (ACT engine, Sign) and w -> +/-0.5 (DVE,
  (v>=0)-0.5, in place), then runs fp8 DoubleRow matmuls (2 virtual PE rows
  per cell = 157 TFLOP/s): products +/-0.5, integer-scaled sums, all exact.
- The PSUM->SBUF eviction copy multiplies by 2 (exact power of two).
  Result is bit-identical to the fp32 reference.

n-chunk 0 runs kt-outer across all 8 PSUM banks so the PE paces behind the
streaming x DMA; later chunks run mt-outer with staggered evictions.
Measured ~248 us/kernel (fp8 roofline for the per-core GEMM is ~219 us).
"""

import numpy as np
import ml_dtypes

N_TOTAL, D_IN, D_OUT = 8192, 4096, 4096
N_CORES = 8
N_PER = N_TOTAL // N_CORES


def fp8_in(mode):
    return mode == "fp8dr"

# "bf16": bf16 matmul (exact).  "fp8dr": fp8e4 DoubleRow matmul (exact, 2
# virtual PE rows per cell -> half the matmul instructions).
MODE = "fp8dr"

_PROGRAM_CACHE = {}


def build_program(n_per=N_PER, d_in=D_IN, d_out=D_OUT, num_devices=N_CORES,
                  mode=None):
    """Build + compile the SPMD Bass program (same program on every core)."""
    from concourse import bacc, mybir, tile
    from concourse.bass import ds

    if mode is None:
        mode = MODE
    BF = mybir.dt.bfloat16
    F32 = mybir.dt.float32
    FP8 = mybir.dt.float8e4
    MMDT = FP8 if mode == "fp8dr" else BF  # matmul operand dtype
    P = 128
    NW = 512  # n-chunk width = one PSUM bank of fp32
    KT = d_in // P      # k-tiles
    MT = n_per // P     # m-tiles per core
    NCH = d_out // NW   # n-chunks
    ge = mybir.AluOpType.is_ge
    sub = mybir.AluOpType.subtract
    Copy = mybir.ActivationFunctionType.Copy
    Sign = mybir.ActivationFunctionType.Sign
    perf_mode = mybir.MatmulPerfMode.DoubleRow if mode == "fp8dr" else None
    # Engine for the x binarize in fp8 mode:
    #   "act"    -> ACT Sign, x = +/-1, copy scale 2
    #   "gpsimd" -> GpSimd (v>=0)-0.5, x = +/-0.5, copy scale 4
    # (w is always +/-0.5 on DVE; host patched +/-0 to +/-1 so either
    # binarizer reproduces sign(v) exactly.)
    XBIN = "act"  # "gpsimd" measured 3x slower end-to-end; keep ACT Sign
    if mode == "fp8dr":
        OUT_SCALE = 4.0 if XBIN == "gpsimd" else 2.0
    else:
        OUT_SCALE = 4.0

    nc = bacc.Bacc(
        "TRN2",
        target_bir_lowering=False,
        debug=False,
        enable_asserts=False,
        num_devices=num_devices,
    )
    # fp8 mode ships inputs as fp8e4 (host re-encode is sign-exact; see
    # shard_inputs) -> half the HBM traffic of bf16.
    INDT = FP8 if fp8_in(mode) else BF
    xt = nc.declare_dram_parameter("xt", [d_in, n_per], INDT, isOutput=False)
    w = nc.declare_dram_parameter("w", [d_in, d_out], INDT, isOutput=False)
    out = nc.declare_dram_parameter("out", [n_per, d_out], F32, isOutput=True)

    # HBM-side access patterns with the k-tile index folded into partitions.
    xt_r = xt.ap().rearrange("(kt p) m -> p kt m", p=P)        # [128, KT, n_per]
    w_r = w.ap().rearrange("(kt p) n -> p kt n", p=P)          # [128, KT, d_out]

    fp8 = mode == "fp8dr"
    if fp8:
        assert KT % 2 == 0

    with tile.TileContext(nc) as tc:
        with (
            tc.tile_pool(name="xpool", bufs=1) as xpool,
            tc.tile_pool(name="wpool", bufs=4 if fp8 else 2) as wpool,
            tc.tile_pool(name="opool", bufs=8) as opool,
            tc.tile_pool(name="psum", bufs=8, space="PSUM") as pspool,
        ):
            xb = xpool.tile([P, KT * n_per], MMDT, tag="xb")
            xb3 = xb[:, :].rearrange("p (kt m) -> p kt m", kt=KT)
            X_CH = min(16, KT)
            kt_per = KT // X_CH

            def x_dma(c, issue_engine=None):
                ktsl = ds(c * kt_per, kt_per)
                eng = issue_engine if issue_engine is not None else nc.sync
                eng.dma_start(out=xb3[:, ktsl, :], in_=xt_r[:, ktsl, :])

            def x_bin(c):
                fsl = ds(c * kt_per * n_per, kt_per * n_per)
                if fp8 and XBIN == "act":
                    # ACT engine; host patched exact zeros so Sign == (v>=0)
                    nc.scalar.activation(xb[:, fsl], xb[:, fsl], Sign)
                elif fp8 and XBIN == "gpsimd":
                    nc.gpsimd.tensor_scalar(
                        xb[:, fsl], xb[:, fsl], 0.0, 0.5, ge, sub
                    )
                else:
                    nc.vector.tensor_scalar(
                        xb[:, fsl], xb[:, fsl], 0.0, 0.5, ge, sub
                    )

            def load_x_chunk(c, issue_engine=None):
                x_dma(c, issue_engine)
                x_bin(c)

            HALF = max(1, KT // 2)
            N_HALVES = KT // HALF
            BIN_KT = min(4, HALF)  # k-tiles per DVE binarize op

            def load_w_chunk(nt, half, n_dmas=1):
                """DMA + binarize (in place) one k-half of w n-chunk nt."""
                nsl = ds(nt * NW, NW)
                wb = w_tiles[nt]
                wb3 = wb[:, :].rearrange("p (kt n) -> p kt n", kt=KT)
                n_dmas = min(n_dmas, HALF)
                per = HALF // n_dmas
                for d in range(n_dmas):
                    hsl = ds(half * HALF + d * per, per)
                    nc.sync.dma_start(
                        out=wb3[:, hsl, :], in_=w_r[:, hsl, nsl]
                    )
                for c in range(HALF // BIN_KT):
                    sl = ds((half * HALF + c * BIN_KT) * NW, BIN_KT * NW)
                    nc.vector.tensor_scalar(
                        wb[:, sl], wb[:, sl], 0.0, 0.5, ge, sub
                    )

            def alloc_w_tiles(nt):
                wb = wpool.tile([P, KT * NW], MMDT, tag="wb", name=f"wb{nt}")
                w_tiles[nt] = wb

            def mm(ps, mt, t, wb3, start, stop):
                if fp8:
                    nc.tensor.matmul(
                        ps[:, :],
                        lhsT=xb3[:, 2 * t : 2 * t + 2, ds(mt * P, P)],
                        rhs=wb3[:, 2 * t : 2 * t + 2, :],
                        start=start, stop=stop, perf_mode=perf_mode,
                    )
                else:
                    nc.tensor.matmul(
                        ps[:, :],
                        lhsT=xb[:, ds(t * n_per + mt * P, P)],
                        rhs=wb3[:, t, :],
                        start=start, stop=stop,
                    )

            def evict(ps, mt, nt, slices=1):
                # slices>1 pipelines copy+DMA in column strips; used for the
                # kernel's final eviction so the exit barrier waits on a
                # short 64KB DMA instead of the full 256KB one.
                ot = opool.tile([P, NW], F32, tag="ot")
                sw = NW // slices
                for i in range(slices):
                    csl = ds(i * sw, sw)
                    nc.scalar.activation(
                        ot[:, csl], ps[:, csl], Copy, 0.0, OUT_SCALE
                    )
                    nc.sync.dma_start(
                        out=out[ds(mt * P, P), ds(nt * NW + i * sw, sw)],
                        in_=ot[:, csl],
                    )

            w_tiles = {}
            NK = KT // 2 if fp8 else KT  # MM k-iterations per psum group

            ps0 = [
                pspool.tile([P, NW], F32, tag="ps", name=f"ps0_{i}")
                for i in range(MT)
            ]

            # HAM warmup: the PE is idle for ~8us while the first DMAs land,
            # and the activity monitor keeps a cold PE at half clock for the
            # first ~3.4us of work. Burn that idle time with dummy matmuls on
            # a memset tile (into ps0[0], which the real k-group overwrites
            # with start=True) so real matmuls start at full clock.
            WARM_MMS = 80 if KT >= 16 else 8
            if WARM_MMS:
                warm = xpool.tile([P, P], MMDT, tag="warm", name="warm")
                nc.gpsimd.memset(warm[:, :], 1.0)
                for _ in range(WARM_MMS):
                    nc.tensor.matmul(
                        ps0[0][:, :P], lhsT=warm[:, :], rhs=warm[:, :],
                        start=True, stop=True,
                    )

            # Startup interleave: first half of w chunk 0, then x, then the
            # rest of w chunk 0 — so the PE can start at the first x k-tiles
            # and never waits on the second w half.
            # Startup interleave: first half of w chunk 0, then x, then the
            # rest of w chunk 0 — so the PE can start at the first x k-tiles
            # and never waits on the second w half. (Offloading x-DMA issues
            # to ACT's queue was tried and measured ~4.5us WORSE: it delays
            # the Sign chain more than it relieves the sync sequencer.)
            alloc_w_tiles(0)
            load_x_chunk(0)
            load_w_chunk(0, 0, n_dmas=2 if HALF >= 8 else 1)
            for c in range(1, X_CH // 2):
                load_x_chunk(c)
            if N_HALVES > 1:
                load_w_chunk(0, 1)
            for c in range(X_CH // 2, X_CH):
                load_x_chunk(c)

            # n-chunk 0: kt-outer across all MT psum banks, pacing the PE
            # behind the streaming x DMA instead of stalling on full x.
            wb3_0 = w_tiles[0][:, :].rearrange("p (kt n) -> p kt n", kt=KT)
            for t in range(NK):
                for mt in range(MT):
                    mm(ps0[mt], mt, t, wb3_0, start=(t == 0), stop=(t == NK - 1))
            for mt in range(MT):
                last = NCH == 1 and mt == MT - 1
                evict(ps0[mt], mt, 0, slices=4 if last and NW >= 512 else 1)

            # n-chunks 1..: mt-outer (staggered psum eviction)
            for nt in range(1, NCH):
                alloc_w_tiles(nt)
                for h in range(N_HALVES):
                    load_w_chunk(nt, h)
                wb3 = w_tiles[nt][:, :].rearrange(
                    "p (kt n) -> p kt n", kt=KT
                )
                for mt in range(MT):
                    ps = pspool.tile([P, NW], F32, tag="ps")
                    for t in range(NK):
                        mm(ps, mt, t, wb3, start=(t == 0), stop=(t == NK - 1))
                    last = nt == NCH - 1 and mt == MT - 1
                    evict(ps, mt, nt, slices=4 if last and NW >= 512 else 1)

    nc.compile()
    return nc


def _get_program():
    key = (N_PER, D_IN, D_OUT, MODE)
    if key not in _PROGRAM_CACHE:
        _PROGRAM_CACHE[key] = build_program()
    return _PROGRAM_CACHE[key]


def _encode_fp8(v):
    """Sign-exact fp8e4 re-encode of fp32 data for the device binarizer.

    ml_dtypes.float8_e4m3 matches TRN FP8_EXP4 (max 240, overflow saturates
    to +/-Inf, underflow to +/-0 -- sign always survives in the result).
    The only sign-ambiguous encodings are +/-0, which we patch to +/-1:
    +0 covers true zeros (reference maps them to +1) and underflowed
    positives; -0 covers underflowed negatives. After the patch the device
    binarize (v >= 0, or Sign) reproduces sign(original fp32) exactly for
    EVERY possible input value.
    """
    f8 = ml_dtypes.float8_e4m3
    v8 = np.clip(v, -240.0, 240.0).astype(f8)
    z = v8 == 0
    if z.any():
        v8 = np.where(z, np.where(np.signbit(v8), -1.0, 1.0).astype(f8), v8)
    return v8


def shard_inputs(x, weight):
    """Host-side sharding/layout: dtype re-encode + per-shard transpose."""
    if fp8_in(MODE):
        xe = _encode_fp8(x)
        we = _encode_fp8(weight)
    else:
        bf16 = ml_dtypes.bfloat16
        xe = x.astype(bf16)
        we = weight.astype(bf16)
    we = np.ascontiguousarray(we)
    shards = [
        np.ascontiguousarray(xe[i * N_PER : (i + 1) * N_PER].T)
        for i in range(N_CORES)
    ]
    return [{"xt": shards[i], "w": we} for i in range(N_CORES)]


def kernel(x, weight):
    from concourse.bass_utils import run_bass_kernel_spmd

    nc = _get_program()
    in_maps = shard_inputs(np.asarray(x), np.asarray(weight))
    res = run_bass_kernel_spmd(nc, in_maps, list(range(N_CORES)))
    return np.concatenate(
        [res.results[i]["out"] for i in range(N_CORES)], axis=0
    )



# revision 3
# speedup vs baseline: 1.0047x; 1.0047x over previous
"""Binary linear layer (sign(x) @ sign(w)) on 8 trn2 NeuronCores.

Strategy
--------
Data-parallel: x is split into 8 row-blocks of 1024; the 4096x4096 weight is
replicated. Each core computes out_shard = sign(x_shard) @ sign(w).

All products are +/-1 and row sums are integers <= 4096, so the matmul is
exact in low precision with fp32 PSUM accumulation. The host ships both
operands already binarized to +/-1 in fp8e4 (a sign(v) re-encode is exactly
as lossy as the sign-exact fp8 cast it replaces, and fp8 transports +/-1
exactly), with each x shard pre-transposed to [d_in, n_per] so the PE
contraction dim lands on SBUF partitions. 20 MB HBM in per core.

The device program is then a pure fp8 DoubleRow GEMM (2 virtual PE rows per
cell): no on-device binarize chains at all, so the first real matmul is
gated only by the first DMA slices. Outputs are integers, evicted
PSUM->SBUF as fp16 (exact to well past the tolerance; |out| <= 4096,
typical |out| ~200) and DMA'd out as 8 MB instead of 16.

Schedule: warmup matmuls on a memset tile burn the DMA-landing latency at
half clock so the HAM un-throttles before/near the first real matmul;
n-chunk 0 runs kt-outer across all 8 PSUM banks so the PE paces behind the
streaming x DMA; later chunks run mt-outer with staggered evictions. DMA
issues are spread across engines (sync: w, vector: x, scalar: out) so no
single sequencer serializes the startup.

Measured ~246 us for the previous on-device-binarize version; the matmul
stream itself is 1024 DR matmuls x ~216 ns = ~221 us, which is the PE
streaming floor for this shape.
"""

import numpy as np
import ml_dtypes

N_TOTAL, D_IN, D_OUT = 8192, 4096, 4096
N_CORES = 8
N_PER = N_TOTAL // N_CORES

_PROGRAM_CACHE = {}


def build_program(n_per=N_PER, d_in=D_IN, d_out=D_OUT, num_devices=N_CORES):
    """Build + compile the SPMD Bass program (same program on every core)."""
    from concourse import bacc, mybir, tile
    from concourse.bass import ds

    F32 = mybir.dt.float32
    F16 = mybir.dt.float16
    FP8 = mybir.dt.float8e4
    P = 128
    NW = 512            # n-chunk width = one PSUM bank of fp32
    KT = d_in // P      # k-tiles (32)
    MT = n_per // P     # m-tiles per core (8)
    NCH = d_out // NW   # n-chunks (8)
    NK = KT // 2        # DR matmuls per accumulation group (16)
    Copy = mybir.ActivationFunctionType.Copy
    perf_mode = mybir.MatmulPerfMode.DoubleRow

    nc = bacc.Bacc(
        "TRN2",
        target_bir_lowering=False,
        debug=False,
        enable_asserts=False,
        num_devices=num_devices,
    )
    xt = nc.declare_dram_parameter("xt", [d_in, n_per], FP8, isOutput=False)
    w = nc.declare_dram_parameter("w", [d_in, d_out], FP8, isOutput=False)
    out = nc.declare_dram_parameter("out", [n_per, d_out], F16, isOutput=True)

    # HBM-side access patterns with the k-tile index folded into partitions.
    xt_r = xt.ap().rearrange("(kt p) m -> p kt m", p=P)        # [128, KT, n_per]
    w_r = w.ap().rearrange("(kt p) n -> p kt n", p=P)          # [128, KT, d_out]

    with tile.TileContext(nc) as tc:
        with (
            tc.tile_pool(name="xpool", bufs=1) as xpool,
            tc.tile_pool(name="wpool", bufs=4) as wpool,
            tc.tile_pool(name="opool", bufs=8) as opool,
            tc.tile_pool(name="psum", bufs=8, space="PSUM") as pspool,
        ):
            xb = xpool.tile([P, KT * n_per], FP8, tag="xb")
            xb3 = xb[:, :].rearrange("p (kt m) -> p kt m", kt=KT)

            # k-tile slice boundaries for the startup DMAs: fine-grained at
            # the front so the first matmuls are gated on the smallest
            # possible transfer, coarse at the back to keep issue count low.
            X_SLICES = [(0, 2), (2, 2), (4, 4), (8, 8), (16, 8), (24, 8)]
            W0_SLICES = [(0, 2), (2, 2), (4, 4), (8, 8), (16, 8), (24, 8)]

            def x_dma(lo, n):
                # gpsimd issues x so the sync queue is free for w: DMA
                # initiation is only legal from gpsimd / SP / Activation.
                ktsl = ds(lo, n)
                nc.gpsimd.dma_start(out=xb3[:, ktsl, :], in_=xt_r[:, ktsl, :])

            w_tiles = {}

            def alloc_w(nt):
                w_tiles[nt] = wpool.tile(
                    [P, KT * NW], FP8, tag="wb", name=f"wb{nt}"
                )

            def load_w(nt, lo, n):
                nsl = ds(nt * NW, NW)
                wb3 = w_tiles[nt][:, :].rearrange("p (kt n) -> p kt n", kt=KT)
                hsl = ds(lo, n)
                nc.sync.dma_start(out=wb3[:, hsl, :], in_=w_r[:, hsl, nsl])

            def mm(ps, mt, t, wb3, start, stop):
                nc.tensor.matmul(
                    ps[:, :],
                    lhsT=xb3[:, 2 * t : 2 * t + 2, ds(mt * P, P)],
                    rhs=wb3[:, 2 * t : 2 * t + 2, :],
                    start=start, stop=stop, perf_mode=perf_mode,
                )

            def evict(ps, mt, nt, slices=1):
                # ACT copies PSUM fp32 -> SBUF fp16 (values are integers
                # <= 4096: exact to ~5e-4 worst case), then the out DMA is
                # issued from the scalar queue right behind the copy.
                ot = opool.tile([P, NW], F16, tag="ot")
                sw = NW // slices
                for i in range(slices):
                    csl = ds(i * sw, sw)
                    nc.scalar.activation(ot[:, csl], ps[:, csl], Copy, 0.0, 1.0)
                    nc.scalar.dma_start(
                        out=out[ds(mt * P, P), ds(nt * NW + i * sw, sw)],
                        in_=ot[:, csl],
                    )

            # HAM warmup: dummy matmuls on a memset tile burn the PE-idle
            # time while the first DMA slices land, so the activity monitor
            # un-throttles the PE clock before the real stream begins. They
            # write into ps0[0], which the real k-group overwrites with
            # start=True.
            ps0 = [
                pspool.tile([P, NW], F32, tag="ps", name=f"ps0_{i}")
                for i in range(MT)
            ]
            WARM_MMS = 30
            warm = xpool.tile([P, P], FP8, tag="warm", name="warm")
            nc.gpsimd.memset(warm[:, :], 1.0)
            for _ in range(WARM_MMS):
                nc.tensor.matmul(
                    ps0[0][:, :P], lhsT=warm[:, :], rhs=warm[:, :],
                    start=True, stop=True,
                )

            # Startup DMAs: w chunk 0 (sync queue) and x (vector queue)
            # issue in parallel, smallest-first, interleaved by need-time.
            alloc_w(0)
            load_w(0, *W0_SLICES[0])
            x_dma(*X_SLICES[0])
            load_w(0, *W0_SLICES[1])
            x_dma(*X_SLICES[1])
            load_w(0, *W0_SLICES[2])
            x_dma(*X_SLICES[2])
            load_w(0, *W0_SLICES[3])
            x_dma(*X_SLICES[3])
            load_w(0, *W0_SLICES[4])
            x_dma(*X_SLICES[4])
            load_w(0, *W0_SLICES[5])
            x_dma(*X_SLICES[5])

            # n-chunk 0: kt-outer across all MT psum banks, pacing the PE
            # behind the streaming x DMA instead of stalling on full x.
            wb3_0 = w_tiles[0][:, :].rearrange("p (kt n) -> p kt n", kt=KT)
            for t in range(NK):
                for mt in range(MT):
                    mm(ps0[mt], mt, t, wb3_0, start=(t == 0), stop=(t == NK - 1))
            for mt in range(MT):
                evict(ps0[mt], mt, 0)

            # n-chunks 1..: mt-outer (staggered psum eviction)
            for nt in range(1, NCH):
                alloc_w(nt)
                load_w(nt, 0, KT // 2)
                load_w(nt, KT // 2, KT // 2)
                wb3 = w_tiles[nt][:, :].rearrange(
                    "p (kt n) -> p kt n", kt=KT
                )
                for mt in range(MT):
                    ps = pspool.tile([P, NW], F32, tag="ps")
                    for t in range(NK):
                        mm(ps, mt, t, wb3, start=(t == 0), stop=(t == NK - 1))
                    last = nt == NCH - 1 and mt == MT - 1
                    evict(ps, mt, nt, slices=2 if last else 1)

    nc.compile()
    return nc


def _get_program():
    key = (N_PER, D_IN, D_OUT)
    if key not in _PROGRAM_CACHE:
        _PROGRAM_CACHE[key] = build_program()
    return _PROGRAM_CACHE[key]


def shard_inputs(x, weight):
    """Host-side sharding/layout: binarize to +/-1 fp8 + per-shard transpose.

    sign semantics match the reference exactly: v >= 0 -> +1 (including
    +/-0.0), else -1. fp8e4m3 represents +/-1 exactly, so the device GEMM
    is bit-exact integer arithmetic in fp32 PSUM.
    """
    f8 = ml_dtypes.float8_e4m3
    one = np.float32(1.0)
    xe = np.where(np.asarray(x) >= 0, one, -one).astype(f8)
    we = np.where(np.asarray(weight) >= 0, one, -one).astype(f8)
    we = np.ascontiguousarray(we)
    shards = [
        np.ascontiguousarray(xe[i * N_PER : (i + 1) * N_PER].T)
        for i in range(N_CORES)
    ]
    return [{"xt": shards[i], "w": we} for i in range(N_CORES)]


def kernel(x, weight):
    from concourse.bass_utils import run_bass_kernel_spmd

    nc = _get_program()
    in_maps = shard_inputs(np.asarray(x), np.asarray(weight))
    res = run_bass_kernel_spmd(nc, in_maps, list(range(N_CORES)))
    return np.concatenate(
        [res.results[i]["out"] for i in range(N_CORES)], axis=0
    ).astype(np.float32)


# revision 7
# speedup vs baseline: 1.0186x; 1.0138x over previous
"""Binary linear layer (sign(x) @ sign(w)) on 8 trn2 NeuronCores.

Strategy
--------
Data-parallel: x is split into 8 row-blocks of 1024; the 4096x4096 weight is
replicated. Each core computes out_shard = sign(x_shard) @ sign(w).

All products are +/-1 and row sums are integers <= 4096, so the matmul is
exact in low precision with fp32 PSUM accumulation. The host ships both
operands already binarized to +/-1 in fp8e4 (a sign(v) re-encode is exactly
as lossy as the sign-exact fp8 cast it replaces, and fp8 transports +/-1
exactly), with each x shard pre-transposed to [d_in, n_per] so the PE
contraction dim lands on SBUF partitions. 20 MB HBM in per core.

The device program is then a pure fp8 DoubleRow GEMM (2 virtual PE rows per
cell): no on-device binarize chains at all, so the first real matmul is
gated only by the first DMA slices. Outputs are integers, evicted
PSUM->SBUF as fp16 (exact to well past the tolerance; |out| <= 4096,
typical |out| ~200) and DMA'd out as 8 MB instead of 16.

Schedule: warmup matmuls on a memset tile burn the DMA-landing latency at
half clock so the HAM un-throttles before/near the first real matmul;
n-chunk 0 runs kt-outer across all 8 PSUM banks so the PE paces behind the
streaming x DMA; later chunks run mt-outer with staggered evictions. DMA
issues are spread across engines (sync: w, vector: x, scalar: out) so no
single sequencer serializes the startup.

Measured ~246 us for the previous on-device-binarize version; the matmul
stream itself is 1024 DR matmuls x ~216 ns = ~221 us, which is the PE
streaming floor for this shape.
"""

import numpy as np
import ml_dtypes

N_TOTAL, D_IN, D_OUT = 8192, 4096, 4096
N_CORES = 8
N_PER = N_TOTAL // N_CORES

_PROGRAM_CACHE = {}


def build_program(n_per=N_PER, d_in=D_IN, d_out=D_OUT, num_devices=N_CORES):
    """Build + compile the SPMD Bass program (same program on every core)."""
    from concourse import bacc, mybir, tile
    from concourse.bass import ds

    F32 = mybir.dt.float32
    F16 = mybir.dt.float16
    FP8 = mybir.dt.float8e4
    P = 128
    NW = 512            # n-chunk width = one PSUM bank of fp32
    KT = d_in // P      # k-tiles (32)
    MT = n_per // P     # m-tiles per core (8)
    NCH = d_out // NW   # n-chunks (8)
    NK = KT // 2        # DR matmuls per accumulation group (16)
    Copy = mybir.ActivationFunctionType.Copy
    perf_mode = mybir.MatmulPerfMode.DoubleRow

    nc = bacc.Bacc(
        "TRN2",
        target_bir_lowering=False,
        debug=False,
        enable_asserts=False,
        num_devices=num_devices,
    )
    xt = nc.declare_dram_parameter("xt", [d_in, n_per], FP8, isOutput=False)
    w = nc.declare_dram_parameter("w", [d_in, d_out], FP8, isOutput=False)
    out = nc.declare_dram_parameter("out", [n_per, d_out], F16, isOutput=True)

    # HBM-side access patterns with the k-tile index folded into partitions.
    xt_r = xt.ap().rearrange("(kt p) m -> p kt m", p=P)        # [128, KT, n_per]
    w_r = w.ap().rearrange("(kt p) n -> p kt n", p=P)          # [128, KT, d_out]

    with tile.TileContext(nc) as tc:
        with (
            tc.tile_pool(name="xpool", bufs=1) as xpool,
            tc.tile_pool(name="wpool", bufs=4) as wpool,
            tc.tile_pool(name="opool", bufs=8) as opool,
            tc.tile_pool(name="psum", bufs=8, space="PSUM") as pspool,
        ):
            xb = xpool.tile([P, KT * n_per], FP8, tag="xb")
            xb3 = xb[:, :].rearrange("p (kt m) -> p kt m", kt=KT)

            # k-tile slice boundaries for the startup DMAs: fine-grained at
            # the front so the first matmuls are gated on the smallest
            # possible transfer, coarse at the back to keep issue count low.
            # Only sync (SP) and scalar (Activation) have hardware DGEs —
            # gpsimd DMA initiation goes through the slow software path.
            def x_dma(lo, n, eng):
                ktsl = ds(lo, n)
                eng.dma_start(out=xb3[:, ktsl, :], in_=xt_r[:, ktsl, :])

            w_tiles = {}

            def alloc_w(nt):
                w_tiles[nt] = wpool.tile(
                    [P, KT * NW], FP8, tag="wb", name=f"wb{nt}"
                )

            def load_w(nt, lo, n, eng=None):
                nsl = ds(nt * NW, NW)
                wb3 = w_tiles[nt][:, :].rearrange("p (kt n) -> p kt n", kt=KT)
                hsl = ds(lo, n)
                (eng or nc.sync).dma_start(out=wb3[:, hsl, :], in_=w_r[:, hsl, nsl])

            def mm(ps, mt, t, wb3, start, stop):
                nc.tensor.matmul(
                    ps[:, :],
                    lhsT=xb3[:, 2 * t : 2 * t + 2, ds(mt * P, P)],
                    rhs=wb3[:, 2 * t : 2 * t + 2, :],
                    start=start, stop=stop, perf_mode=perf_mode,
                )

            def evict(ps, mt, nt, last=False):
                # ACT copies PSUM fp32 -> SBUF fp16 (values are integers
                # <= 4096: exact to ~5e-4 worst case), then the out DMA is
                # issued from the scalar queue right behind the copy. The
                # kernel's final eviction splits into two column strips with
                # the issues on sync+scalar so copy/issue/transfer pipeline
                # and the exit barrier waits on the smallest possible DMA.
                ot = opool.tile([P, NW], F16, tag="ot")
                if not last:
                    nc.scalar.activation(ot[:, :], ps[:, :], Copy, 0.0, 1.0)
                    nc.scalar.dma_start(
                        out=out[ds(mt * P, P), ds(nt * NW, NW)], in_=ot[:, :]
                    )
                    return
                half = NW // 2
                for i, eng in ((0, nc.sync), (1, nc.scalar)):
                    csl = ds(i * half, half)
                    nc.scalar.activation(ot[:, csl], ps[:, csl], Copy, 0.0, 1.0)
                    eng.dma_start(
                        out=out[ds(mt * P, P), ds(nt * NW + i * half, half)],
                        in_=ot[:, csl],
                    )

            # HAM warmup: dummy matmuls on a memset tile burn the PE-idle
            # time while the first DMA slices land, so the activity monitor
            # un-throttles the PE clock before the real stream begins. They
            # write into ps0[0], which the real k-group overwrites with
            # start=True.
            ps0 = [
                pspool.tile([P, NW], F32, tag="ps", name=f"ps0_{i}")
                for i in range(MT)
            ]
            WARM_MMS = 30
            warm = xpool.tile([P, P], FP8, tag="warm", name="warm")
            nc.gpsimd.memset(warm[:, :], 1.0)
            for _ in range(WARM_MMS):
                nc.tensor.matmul(
                    ps0[0][:, :P], lhsT=warm[:, :], rhs=warm[:, :],
                    start=True, stop=True,
                )

            # Startup DMAs, interleaved by need-time across the two HW DGE
            # queues. sync: x/w slices that gate the first matmuls, then the
            # rest of w chunk 0. scalar: the later x slices (behind the
            # auto-inserted ACT table load, which is off the critical path).
            alloc_w(0)
            x_dma(0, 2, nc.sync)       # kt 0-1, gates t=0
            load_w(0, 0, 2)            # kt 0-1, gates t=0
            x_dma(2, 2, nc.scalar)     # kt 2-3, gates t=1
            load_w(0, 2, 2)            # kt 2-3
            x_dma(4, 4, nc.scalar)     # kt 4-7
            load_w(0, 4, 4)
            x_dma(8, 8, nc.scalar)
            load_w(0, 8, 8)
            x_dma(16, 8, nc.scalar)
            load_w(0, 16, 8)
            x_dma(24, 8, nc.scalar)
            load_w(0, 24, 8)

            # n-chunk 0: kt-outer across all MT psum banks, pacing the PE
            # behind the streaming x DMA instead of stalling on full x.
            wb3_0 = w_tiles[0][:, :].rearrange("p (kt n) -> p kt n", kt=KT)
            for t in range(NK):
                for mt in range(MT):
                    mm(ps0[mt], mt, t, wb3_0, start=(t == 0), stop=(t == NK - 1))
            for mt in range(MT):
                evict(ps0[mt], mt, 0)

            # n-chunks 1..: mt-outer (staggered psum eviction)
            for nt in range(1, NCH):
                alloc_w(nt)
                load_w(nt, 0, KT // 2)
                load_w(nt, KT // 2, KT // 2)
                wb3 = w_tiles[nt][:, :].rearrange(
                    "p (kt n) -> p kt n", kt=KT
                )
                for mt in range(MT):
                    ps = pspool.tile([P, NW], F32, tag="ps")
                    for t in range(NK):
                        mm(ps, mt, t, wb3, start=(t == 0), stop=(t == NK - 1))
                    evict(ps, mt, nt, last=(nt == NCH - 1 and mt == MT - 1))

    nc.compile()
    return nc


def _get_program():
    key = (N_PER, D_IN, D_OUT)
    if key not in _PROGRAM_CACHE:
        _PROGRAM_CACHE[key] = build_program()
    return _PROGRAM_CACHE[key]


def shard_inputs(x, weight):
    """Host-side sharding/layout: binarize to +/-1 fp8 + per-shard transpose.

    sign semantics match the reference exactly: v >= 0 -> +1 (including
    +/-0.0), else -1. fp8e4m3 represents +/-1 exactly, so the device GEMM
    is bit-exact integer arithmetic in fp32 PSUM.
    """
    f8 = ml_dtypes.float8_e4m3
    one = np.float32(1.0)
    xe = np.where(np.asarray(x) >= 0, one, -one).astype(f8)
    we = np.where(np.asarray(weight) >= 0, one, -one).astype(f8)
    we = np.ascontiguousarray(we)
    shards = [
        np.ascontiguousarray(xe[i * N_PER : (i + 1) * N_PER].T)
        for i in range(N_CORES)
    ]
    return [{"xt": shards[i], "w": we} for i in range(N_CORES)]


def kernel(x, weight):
    from concourse.bass_utils import run_bass_kernel_spmd

    nc = _get_program()
    in_maps = shard_inputs(np.asarray(x), np.asarray(weight))
    res = run_bass_kernel_spmd(nc, in_maps, list(range(N_CORES)))
    return np.concatenate(
        [res.results[i]["out"] for i in range(N_CORES)], axis=0
    ).astype(np.float32)


# revision 9
# speedup vs baseline: 1.0316x; 1.0128x over previous
"""Binary linear layer (sign(x) @ sign(w)) on 8 trn2 NeuronCores.

Strategy
--------
Data-parallel: x is split into 8 row-blocks of 1024; the 4096x4096 weight is
replicated. Each core computes out_shard = sign(x_shard) @ sign(w).

All products are +/-1 and row sums are integers <= 4096, so the matmul is
exact in low precision with fp32 PSUM accumulation. The host ships both
operands already binarized to +/-1 in fp8e4 (a sign(v) re-encode is exactly
as lossy as the sign-exact fp8 cast it replaces, and fp8 transports +/-1
exactly), with each x shard pre-transposed to [d_in, n_per] so the PE
contraction dim lands on SBUF partitions. 20 MB HBM in per core.

The device program is then a pure fp8 DoubleRow GEMM (2 virtual PE rows per
cell): no on-device binarize chains at all, so the first real matmul is
gated only by the first DMA slices. Outputs are integers, evicted
PSUM->SBUF as fp16 (exact to well past the tolerance; |out| <= 4096,
typical |out| ~200) and DMA'd out as 8 MB instead of 16.

Schedule: warmup matmuls on a memset tile burn the DMA-landing latency at
half clock so the HAM un-throttles before/near the first real matmul;
n-chunk 0 runs kt-outer across all 8 PSUM banks so the PE paces behind the
streaming x DMA; later chunks run mt-outer with staggered evictions. DMA
issues are spread across engines (sync: w, vector: x, scalar: out) so no
single sequencer serializes the startup.

Measured ~246 us for the previous on-device-binarize version; the matmul
stream itself is 1024 DR matmuls x ~216 ns = ~221 us, which is the PE
streaming floor for this shape.
"""

import numpy as np
import ml_dtypes

N_TOTAL, D_IN, D_OUT = 8192, 4096, 4096
N_CORES = 8
N_PER = N_TOTAL // N_CORES

_PROGRAM_CACHE = {}


def build_program(n_per=N_PER, d_in=D_IN, d_out=D_OUT, num_devices=N_CORES):
    """Build + compile the SPMD Bass program (same program on every core)."""
    from concourse import bacc, mybir, tile
    from concourse.bass import ds

    F32 = mybir.dt.float32
    F16 = mybir.dt.float16
    FP8 = mybir.dt.float8e4
    P = 128
    NW = 512            # n-chunk width = one PSUM bank of fp32
    KT = d_in // P      # k-tiles (32)
    MT = n_per // P     # m-tiles per core (8)
    NCH = d_out // NW   # n-chunks (8)
    NK = KT // 2        # DR matmuls per accumulation group (16)
    Copy = mybir.ActivationFunctionType.Copy
    perf_mode = mybir.MatmulPerfMode.DoubleRow

    nc = bacc.Bacc(
        "TRN2",
        target_bir_lowering=False,
        debug=False,
        enable_asserts=False,
        num_devices=num_devices,
    )
    xt = nc.declare_dram_parameter("xt", [d_in, n_per], FP8, isOutput=False)
    w = nc.declare_dram_parameter("w", [d_in, d_out], FP8, isOutput=False)
    out = nc.declare_dram_parameter("out", [n_per, d_out], F16, isOutput=True)

    # HBM-side access patterns with the k-tile index folded into partitions.
    xt_r = xt.ap().rearrange("(kt p) m -> p kt m", p=P)        # [128, KT, n_per]
    w_r = w.ap().rearrange("(kt p) n -> p kt n", p=P)          # [128, KT, d_out]

    with tile.TileContext(nc) as tc:
        with (
            tc.tile_pool(name="xpool", bufs=1) as xpool,
            tc.tile_pool(name="wpool", bufs=4) as wpool,
            tc.tile_pool(name="opool", bufs=8) as opool,
            tc.tile_pool(name="psum", bufs=8, space="PSUM") as pspool,
        ):
            xb = xpool.tile([P, KT * n_per], FP8, tag="xb")
            xb3 = xb[:, :].rearrange("p (kt m) -> p kt m", kt=KT)

            # k-tile slice boundaries for the startup DMAs: fine-grained at
            # the front so the first matmuls are gated on the smallest
            # possible transfer, coarse at the back to keep issue count low.
            # Only sync (SP) and scalar (Activation) have hardware DGEs —
            # gpsimd DMA initiation goes through the slow software path.
            def x_dma(lo, n, eng):
                ktsl = ds(lo, n)
                eng.dma_start(out=xb3[:, ktsl, :], in_=xt_r[:, ktsl, :])

            w_tiles = {}

            def alloc_w(nt):
                w_tiles[nt] = wpool.tile(
                    [P, KT * NW], FP8, tag="wb", name=f"wb{nt}"
                )

            def load_w(nt, lo, n, eng=None):
                nsl = ds(nt * NW, NW)
                wb3 = w_tiles[nt][:, :].rearrange("p (kt n) -> p kt n", kt=KT)
                hsl = ds(lo, n)
                (eng or nc.sync).dma_start(out=wb3[:, hsl, :], in_=w_r[:, hsl, nsl])

            def mm(ps, mt, t, wb3, start, stop):
                nc.tensor.matmul(
                    ps[:, :],
                    lhsT=xb3[:, 2 * t : 2 * t + 2, ds(mt * P, P)],
                    rhs=wb3[:, 2 * t : 2 * t + 2, :],
                    start=start, stop=stop, perf_mode=perf_mode,
                )

            def evict(ps, mt, nt, last=False):
                # ACT copies PSUM fp32 -> SBUF fp16 (values are integers
                # <= 4096: exact to ~5e-4 worst case), then the out DMA is
                # issued from the scalar queue right behind the copy. The
                # kernel's final eviction splits into two column strips with
                # the issues on sync+scalar so copy/issue/transfer pipeline
                # and the exit barrier waits on the smallest possible DMA.
                ot = opool.tile([P, NW], F16, tag="ot")
                if not last:
                    nc.scalar.activation(ot[:, :], ps[:, :], Copy, 0.0, 1.0)
                    nc.scalar.dma_start(
                        out=out[ds(mt * P, P), ds(nt * NW, NW)], in_=ot[:, :]
                    )
                    return
                half = NW // 2
                for i, eng in ((0, nc.sync), (1, nc.scalar)):
                    csl = ds(i * half, half)
                    nc.scalar.activation(ot[:, csl], ps[:, csl], Copy, 0.0, 1.0)
                    eng.dma_start(
                        out=out[ds(mt * P, P), ds(nt * NW + i * half, half)],
                        in_=ot[:, csl],
                    )

            # HAM warmup: dummy matmuls on a memset tile burn the PE-idle
            # time while the first DMA slices land, so the activity monitor
            # un-throttles the PE clock before the real stream begins. They
            # write into ps0[0], which the real k-group overwrites with
            # start=True.
            ps0 = [
                pspool.tile([P, NW], F32, tag="ps", name=f"ps0_{i}")
                for i in range(MT)
            ]
            WARM_MMS = 40
            warm = xpool.tile([P, P], FP8, tag="warm", name="warm")
            nc.gpsimd.memset(warm[:, :], 1.0)
            for _ in range(WARM_MMS):
                nc.tensor.matmul(
                    ps0[0][:, :P], lhsT=warm[:, :], rhs=warm[:, :],
                    start=True, stop=True,
                )

            # Startup DMAs: everything on the sync HW-DGE queue in strict
            # need-order (t-row r consumes k-tile pair 2r,2r+1 every
            # ~1.73us), fine slices at the front so the first matmuls gate
            # on the smallest possible transfer. A second queue would let
            # far-future slices jump ahead of critical early ones; supply
            # rate (~300 GB/s) exceeds demand (~220 GB/s), so one strictly
            # ordered queue is stall-free once the first pair lands.
            alloc_w(0)
            for lo, n in ((0, 2), (2, 2), (4, 2), (6, 2),
                          (8, 4), (12, 4), (16, 4), (20, 4), (24, 4), (28, 4)):
                x_dma(lo, n, nc.sync)
                load_w(0, lo, n)

            # n-chunk 0: kt-outer across all MT psum banks, pacing the PE
            # behind the streaming x DMA instead of stalling on full x.
            wb3_0 = w_tiles[0][:, :].rearrange("p (kt n) -> p kt n", kt=KT)
            for t in range(NK):
                for mt in range(MT):
                    mm(ps0[mt], mt, t, wb3_0, start=(t == 0), stop=(t == NK - 1))
            for mt in range(MT):
                evict(ps0[mt], mt, 0)

            # n-chunks 1..: mt-outer (staggered psum eviction)
            for nt in range(1, NCH):
                alloc_w(nt)
                load_w(nt, 0, KT // 2)
                load_w(nt, KT // 2, KT // 2)
                wb3 = w_tiles[nt][:, :].rearrange(
                    "p (kt n) -> p kt n", kt=KT
                )
                for mt in range(MT):
                    ps = pspool.tile([P, NW], F32, tag="ps")
                    for t in range(NK):
                        mm(ps, mt, t, wb3, start=(t == 0), stop=(t == NK - 1))
                    evict(ps, mt, nt, last=(nt == NCH - 1 and mt == MT - 1))

    nc.compile()
    return nc


def _get_program():
    key = (N_PER, D_IN, D_OUT)
    if key not in _PROGRAM_CACHE:
        _PROGRAM_CACHE[key] = build_program()
    return _PROGRAM_CACHE[key]


def shard_inputs(x, weight):
    """Host-side sharding/layout: binarize to +/-1 fp8 + per-shard transpose.

    sign semantics match the reference exactly: v >= 0 -> +1 (including
    +/-0.0), else -1. fp8e4m3 represents +/-1 exactly, so the device GEMM
    is bit-exact integer arithmetic in fp32 PSUM.
    """
    f8 = ml_dtypes.float8_e4m3
    one = np.float32(1.0)
    xe = np.where(np.asarray(x) >= 0, one, -one).astype(f8)
    we = np.where(np.asarray(weight) >= 0, one, -one).astype(f8)
    we = np.ascontiguousarray(we)
    shards = [
        np.ascontiguousarray(xe[i * N_PER : (i + 1) * N_PER].T)
        for i in range(N_CORES)
    ]
    return [{"xt": shards[i], "w": we} for i in range(N_CORES)]


def kernel(x, weight):
    from concourse.bass_utils import run_bass_kernel_spmd

    nc = _get_program()
    in_maps = shard_inputs(np.asarray(x), np.asarray(weight))
    res = run_bass_kernel_spmd(nc, in_maps, list(range(N_CORES)))
    return np.concatenate(
        [res.results[i]["out"] for i in range(N_CORES)], axis=0
    ).astype(np.float32)
